# revision 30
# baseline (speedup 1.0000x reference)
"""Distributed multi-head attention kernel for one TRN2 chip (8 NeuronCores).

Problem: y = Attention(x) with b=2, n=2048, dim=1024, heads=16, dim_head=64.

Sharding (data + tensor parallel, per the hint):
  core c:  batch b = c // 4,  head-group r = c % 4  (4 heads = 256 inner dims)
  Host gathers by summing the 4 rank-256 partial y's per batch (the
  tensor-parallel all-reduce) and adds the bias - no on-device collective.

v2 design (all-bf16 matmuls; ~141us of PE work is the roofline):
  - Host pre-transposes x to feature-major chunks, so no on-device
    DMA transposes are needed for the input.
  - scores^T per (i-block, j-tile): k stationary, q moving -> PSUM
    [128 j, 512 iA | 512 iB]; exp on ACT (bf16 out, scale=1/8 inside).
    A small share of exp tiles run on the DVE via the Schraudolph int16
    bit-trick (bits = s*128*log2e + (127*128 - C), bitcast bf16), which
    keeps the ACT stream off the critical path.
  - AV is TRANSPOSED vs v1: e-chunk [128 j, 128 i] is the stationary
    operand, v [128 j, 64] the moving one -> out [128 i, 64] uses the
    full PE width (half the cost of the 65-row layout), and an extra
    1-row matmul per chunk (rhs = ones column) accumulates the softmax
    denominators as PSUM COLUMNS [128 i, 1].
  - Normalization is then one tiny reciprocal [128, 8] plus 8 per-
    partition-scalar multiplies per block (no PE broadcast / rep tiles).
  - aTT [i, inner] is flipped back to aT [inner, i] with cheap PE
    transposes (128 rows each) staged through one shared PSUM bank.
  - Out-projection per token-tile from aT, interleaved into later
    blocks' attention streams; y partials stored f32.
  Block order interleaves head-pairs so aT token-ranges complete early
  and the out-projection spreads instead of piling into the tail.
"""

import sys

if "/opt/trn_rl_repo" not in sys.path:
    sys.path.insert(0, "/opt/trn_rl_repo")

from contextlib import ExitStack

import numpy as np

import concourse.bass as bass
from concourse import bacc
import concourse.mybir as mybir
import concourse.tile as tile
from concourse.masks import make_identity

F32 = mybir.dt.float32
BF16 = mybir.dt.bfloat16
I16 = mybir.dt.int16
EXP = mybir.ActivationFunctionType.Exp

B, N, DIM = 2, 2048, 1024
HEADS, DH = 16, 64
INNER = HEADS * DH            # 1024
SCALE = DH ** -0.5            # 0.125
GROUP = 4                     # tensor-parallel group size (cores per batch)
IC = INNER // GROUP           # 256 inner dims per core (4 heads)
NEG = -1.0e30                 # additive mask bias

P = 128                       # partitions
TB = 512                      # i-block width
NT = N // P                   # 16 token tiles
ND = DIM // P                 # 8 contraction chunks
NTB = N // TB                 # 4 token blocks

LOG2E = 1.4426950408889634
TRICK_C = 7.0                 # Schraudolph bias correction (calibrated)
TRICK_M = SCALE * 128.0 * LOG2E
TRICK_B = 127.0 * 128.0 - TRICK_C

# (block-index -> set of jt) handled by the DVE bit-trick instead of ACT.
# ~5/16 of tiles per block run on the DVE so the two exp streams advance
# in parallel (the psum score slots recycle faster, keeping the PE fed);
# the last block goes half-and-half to shrink the serial tail. Accuracy:
# measured 1.27e-2 end-to-end even at a 64/128 share (C=7).
TRICK_JTS = {0: {0, 5, 10}, 1: {0, 8}, 2: {0, 8}, 3: {0}, 4: {0},
             5: {0}, 6: {0}, 7: {0, 9, 11, 13, 15}}

# Block order: (hp, ib) pairs; hp-interleaved so token ranges finish early.
BLOCKS = [(0, 0), (0, 1), (1, 0), (0, 2), (1, 1), (0, 3), (1, 2), (1, 3)]

_CACHE = {}


def _mm(nc, out, lhsT, rhs, start=True, stop=True, tile_position=None,
        is_transpose=None):
    nc.tensor.matmul(
        out, lhsT, rhs, start=start, stop=stop, tile_position=tile_position,
        is_transpose=is_transpose,
    )


def _build(mask_any: bool) -> bass.Bass:
    nc = bacc.Bacc()

    xt = nc.declare_dram_parameter("xt", [ND, P, N], BF16, False)
    wq = nc.declare_dram_parameter("wq_s", [DIM, IC], BF16, False)
    wk = nc.declare_dram_parameter("wk_s", [DIM, IC], BF16, False)
    wv = nc.declare_dram_parameter("wv_s", [DIM, IC], BF16, False)
    wo = nc.declare_dram_parameter("wo_s", [IC, DIM], BF16, False)
    if mask_any:
        mb = nc.declare_dram_parameter("mbias", [P, NT], F32, False)
    y = nc.declare_dram_parameter("y", [N, DIM], F32, True)

    with ExitStack() as ctx:
        tc = ctx.enter_context(tile.TileContext(nc))

        const = ctx.enter_context(tc.tile_pool(name="const", bufs=1))
        ident_f32 = const.tile([P, P], F32, tag="ident_f32")
        make_identity(nc, ident_f32[:])
        ident_bf = const.tile([P, P], BF16, tag="ident_bf")
        nc.vector.tensor_copy(ident_bf[:], ident_f32[:])
        ones_col = const.tile([P, 1], BF16, tag="ones_col")
        nc.vector.memset(ones_col[:], 1.0)
        zrow = const.tile([1, TB], BF16, tag="zrow")
        nc.vector.memset(zrow[:], 0.0)
        if mask_any:
            mb_sb = const.tile([P, NT], F32, tag="mb_sb")
            nc.sync.dma_start(mb_sb[:], mb.ap())
        warm = const.tile([P, TB], BF16, tag="warm")
        nc.vector.memset(warm[:], 1.0)

        # ---- persistent SBUF ----
        sb = ctx.enter_context(tc.tile_pool(name="sb", bufs=1))
        xT2 = sb.tile([P, ND * N], BF16, tag="xT", name="xT2")
        xT = [xT2[:, c * N: (c + 1) * N] for c in range(ND)]
        wq_sb2 = sb.tile([P, ND * IC], BF16, tag="wq", name="wq_sb2")
        wk_sb2 = sb.tile([P, ND * IC], BF16, tag="wk", name="wk_sb2")
        wv_sb2 = sb.tile([P, ND * IC], BF16, tag="wv", name="wv_sb2")
        wq_sb = [wq_sb2[:, c * IC: (c + 1) * IC] for c in range(ND)]
        wk_sb = [wk_sb2[:, c * IC: (c + 1) * IC] for c in range(ND)]
        wv_sb = [wv_sb2[:, c * IC: (c + 1) * IC] for c in range(ND)]
        wo_sb2 = sb.tile([P, 2 * DIM], BF16, tag="wo", name="wo_sb2")
        wo_sb = [wo_sb2[:, c * DIM: (c + 1) * DIM] for c in range(2)]

        q2 = [sb.tile([P, N], BF16, tag=f"q{hp}", name=f"q2_{hp}")
              for hp in range(2)]
        k2 = [sb.tile([P, N], BF16, tag=f"k{hp}", name=f"k2_{hp}")
              for hp in range(2)]
        # v_ext: head h (0..3) at cols [h*NT*DH, ...); jt chunk at +jt*64.
        v_ext = sb.tile([P, 4 * NT * DH], BF16, tag="vx", name="v_ext")
        # aT2: inner-major chunks per head-pair: chunk hp at cols [hp*N, ...)
        aT2 = sb.tile([P, 2 * N], BF16, tag="aT", name="aT2")

        e_pool = ctx.enter_context(tc.tile_pool(name="ep", bufs=32))
        att = ctx.enter_context(tc.tile_pool(name="att", bufs=1))

        # ---- PSUM: 4 (scores) + 1 (av) + 1 (dcol+pT) + 2 (pj) = 8 banks
        ps_sc = ctx.enter_context(tc.tile_pool(name="ps_sc", bufs=2, space="PSUM"))
        ps_av = ctx.enter_context(tc.tile_pool(name="ps_av", bufs=1, space="PSUM"))
        ps_sm = ctx.enter_context(tc.tile_pool(name="ps_sm", bufs=1, space="PSUM"))
        ps_pj = ctx.enter_context(tc.tile_pool(name="ps_pj", bufs=2, space="PSUM"))

        smalls = ps_sm.tile([P, TB], F32, tag="sm", name="smalls")
        dcol = smalls[:, 0:64]                      # denominator columns
        pT = smalls.bitcast(BF16)[:, 128: 128 + TB]  # transposed-aT staging

        # ---- DMAs (issue everything up front; deps do the sequencing).
        # The prefix is latency-critical: tb0 lands per-chunk so the first
        # projection matmuls start after ~one chunk, and the weights ride
        # the second queue (ACT's HWDGE port is idle at t=0).
        def dma_xt(tg, eng=None):
            (eng or nc.sync).dma_start(
                xT2[:].rearrange("p (c t) -> p c t", c=ND)[
                    :, :, tg * TB: (tg + 1) * TB
                ],
                xt.ap().rearrange("c p t -> p c t")[:, :, tg * TB: (tg + 1) * TB],
            )

        def dma_w(dst2, src, clo, chi, eng):
            eng.dma_start(
                dst2[:].rearrange("p (c i) -> p c i", c=ND)[:, clo:chi],
                src.ap().rearrange("(c p) i -> p c i", c=ND)[:, clo:chi],
            )

        def dma_xt_chunk(tg, c, eng):
            eng.dma_start(
                xT2[:].rearrange("p (c t) -> p c t", c=ND)[
                    :, c: c + 1, tg * TB: (tg + 1) * TB
                ],
                xt.ap()[c: c + 1, :, tg * TB: (tg + 1) * TB].rearrange(
                    "c p t -> p c t"
                ),
            )

        dma_xt(0)
        dma_w(wq_sb2, wq, 0, ND, nc.sync)
        dma_w(wk_sb2, wk, 0, ND, nc.sync)
        dma_xt(1)
        dma_w(wv_sb2, wv, 0, ND, nc.sync)
        dma_xt(2)
        dma_xt(3)
        nc.sync.dma_start(
            wo_sb2[:].rearrange("p (c d) -> p c d", c=2),
            wo.ap().rearrange("(c p) d -> p c d", c=2),
        )

        # PE p-state warm-up gated on the first weight DMA.
        for _ in range(3):
            pw = ps_pj.tile([P, TB], F32, tag="pj", name="pw")
            nc.tensor.matmul(pw[:], ident_bf[:], wq_sb2[:, 0:TB],
                             start=True, stop=True)

        # ---- work units -------------------------------------------------
        def qk_unit(hp, tb, wsb, dest):
            ps = ps_pj.tile([P, TB], F32, tag="pj", name="psqk")
            for c in range(ND):
                _mm(nc, ps[:], wsb[c][:, hp * P: (hp + 1) * P],
                    xT[c][:, tb * TB: (tb + 1) * TB],
                    start=(c == 0), stop=(c == ND - 1))
            nc.vector.tensor_copy(dest[:, tb * TB: (tb + 1) * TB], ps[:])

        v_done = [0]

        def v_unit(t):
            ps = ps_pj.tile([P, TB], F32, tag="pj", name="psv")
            for c in range(ND):
                _mm(nc, ps[:, 0:IC], xT[c][:, t * P: (t + 1) * P],
                    wv_sb[c][:], start=(c == 0), stop=(c == ND - 1))
            nc.vector.tensor_copy(
                v_ext[:].rearrange("p (h jx) -> p h jx", h=4)[
                    :, :, t * DH: (t + 1) * DH
                ],
                ps[:, 0:IC].rearrange("p (h d) -> p h d", h=4),
            )
            v_done[0] += 1

        def outproj_unit(t, nb, fouts):
            psy = ps_pj.tile([P, TB], F32, tag="pj", name="psy")
            for hp in range(2):
                _mm(nc, psy[:], aT2[:, hp * N + t * P: hp * N + (t + 1) * P],
                    wo_sb[hp][:, nb * TB: (nb + 1) * TB],
                    start=(hp == 0), stop=(hp == 1))
            if nb == 0:
                fouts["f"] = att.tile([P, DIM], F32, tag="fout", bufs=4,
                                      name="fout")
            nc.vector.tensor_copy(
                fouts["f"][:, nb * TB: (nb + 1) * TB], psy[:]
            )
            if nb == 1:
                nc.sync.dma_start(
                    y.ap()[t * P: (t + 1) * P, :], fouts["f"][:]
                )

        def outproj_steps(ib):
            out = []
            for t in range(ib * NTB, (ib + 1) * NTB):
                fouts = {}
                for nb in range(2):
                    out.append(lambda t=t, nb=nb, fouts=fouts:
                               outproj_unit(t, nb, fouts))
            return out

        # ---- prefix -----------------------------------------------------
        qk_unit(0, 0, wq_sb, q2[0])
        qk_unit(0, 0, wk_sb, k2[0])

        steps = []
        steps.append(lambda: qk_unit(0, 1, wk_sb, k2[0]))
        steps.append(lambda: v_unit(0))
        steps.append(lambda: qk_unit(0, 2, wk_sb, k2[0]))
        steps.append(lambda: v_unit(1))
        steps.append(lambda: qk_unit(0, 3, wk_sb, k2[0]))
        steps.append(lambda: qk_unit(0, 1, wq_sb, q2[0]))  # blk idx1 = (0,1)
        for tb in range(NTB):
            steps.append(lambda tb=tb: qk_unit(1, tb, wq_sb, q2[1]))
            steps.append(lambda tb=tb: qk_unit(1, tb, wk_sb, k2[1]))
            steps.append(lambda t=tb + 2: v_unit(t))
        steps.append(lambda: qk_unit(0, 2, wq_sb, q2[0]))  # blk idx3 = (0,2)
        for t in range(6, NT):
            steps.append(lambda t=t: v_unit(t))
        steps.append(lambda: qk_unit(0, 3, wq_sb, q2[0]))  # blk idx5 = (0,3)

        # ---- attention --------------------------------------------------
        av_backlog = []   # (blk_idx, jt, e, hp, last)
        norm_q = []       # deferred per-block normalization stages
        av_popped = [0] * len(BLOCKS)

        def scores(hp, i0, jt):
            isl = slice(i0, i0 + TB)
            jsl = slice(jt * P, (jt + 1) * P)
            psAB = ps_sc.tile([P, 2 * TB], F32, tag="sc", name="psAB")
            _mm(nc, psAB[:, 0:TB], k2[hp][0:DH, jsl], q2[hp][0:DH, isl],
                tile_position=(0, 0))
            _mm(nc, psAB[:, TB: 2 * TB], k2[hp][DH:P, jsl], q2[hp][DH:P, isl],
                tile_position=(DH, 0))
            if mask_any:
                mcol = mb_sb[:, jt: jt + 1]
                nc.vector.tensor_scalar_add(psAB[:], psAB[:], mcol)
            return psAB

        def exp_unit(blk_idx, jt, psAB):
            e = e_pool.tile([P, 2 * TB], BF16, tag="e", name="e")
            if jt in TRICK_JTS.get(blk_idx, ()):
                nc.vector.tensor_scalar(
                    e.bitcast(I16)[:], psAB[:], TRICK_M, TRICK_B,
                    mybir.AluOpType.mult, mybir.AluOpType.add,
                )
            else:
                nc.scalar.activation(e[:], psAB[:], EXP, scale=SCALE)
            return e

        def bank_claim(dst):
            # PSUM start=True lazily zeroes the entire 2KB bank, so banks
            # holding several accumulation groups are claimed ONCE by a
            # whole-bank zero write (outer product with a zero row); the
            # real accumulations then run start=False. This also hands the
            # scheduler a full-bank WAR dependency against prior readers.
            nc.tensor.matmul(dst, warm[0:1, 0:P], zrow[0:1, :],
                             start=True, stop=True)

        def av_unit(blk_idx, jt, e, hp, av_t, last):
            if jt == 0:
                bank_claim(av_t[:])
                bank_claim(smalls[:])
            for half in range(2):
                h = 2 * hp + half
                for c in range(4):
                    g = half * 4 + c
                    echunk = e[:, (half * 4 + c) * P: (half * 4 + c + 1) * P]
                    hb = h * NT * DH
                    nc.tensor.matmul(
                        av_t[:, g * DH: (g + 1) * DH], echunk,
                        v_ext[:, hb + jt * DH: hb + (jt + 1) * DH],
                        start=False, stop=last, skip_group_check=True)
                    nc.tensor.matmul(
                        dcol[:, blk_idx * 8 + g: blk_idx * 8 + g + 1],
                        echunk, ones_col[:],
                        start=False, stop=last, skip_group_check=True)
            av_popped[blk_idx] += 1

        def norm_stages(blk_idx, hp, i0, av_t):
            # recip of the block's 8 denominator columns, then 8 per-
            # partition-scalar muls into aTT_sb, then 8 PE transposes into
            # pT, then one drain into aT2.
            rcp = att.tile([P, 8], F32, tag="rcp", bufs=2, name="rcp")
            aTT = att.tile([P, TB], BF16, tag="aTT", bufs=2, name="aTT")
            stages = []

            def s_recip():
                nc.vector.reciprocal(rcp[:], dcol[:, blk_idx * 8: blk_idx * 8 + 8])

            def s_mul(g):
                nc.vector.tensor_scalar(
                    aTT[:, g * DH: (g + 1) * DH],
                    av_t[:, g * DH: (g + 1) * DH],
                    rcp[:, g: g + 1], None, mybir.AluOpType.mult,
                )

            def s_transpose(g):
                half, c = divmod(g, 4)
                nc.tensor.matmul(
                    pT[half * DH: (half + 1) * DH, c * P: (c + 1) * P],
                    aTT[:, g * DH: (g + 1) * DH], ident_bf[:],
                    is_transpose=True, start=False, stop=True,
                    skip_group_check=True,
                    tile_position=(0, half * DH))

            def s_drain():
                nc.vector.tensor_copy(
                    aT2[:, hp * N + i0: hp * N + i0 + TB], pT[:, 0:TB]
                )

            stages.append(s_recip)
            stages.append(lambda: (s_mul(0), s_mul(1), s_mul(2), s_mul(3)))
            stages.append(lambda: (s_mul(4), s_mul(5), s_mul(6), s_mul(7)))
            stages.append(lambda: (s_transpose(0), s_transpose(1),
                                   s_transpose(2), s_transpose(3)))
            stages.append(lambda: (s_transpose(4), s_transpose(5),
                                   s_transpose(6), s_transpose(7)))
            stages.append(s_drain)
            return stages

        def pop_av():
            if not av_backlog:
                return False
            blk_idx, jt, e, hp, av_t, last = av_backlog[0]
            if v_done[0] <= jt:
                return False
            if jt == 0 and norm_q:
                # av_t bank is recycled (ring=1): the new block's first AV
                # must not be emitted on the PE before the previous block's
                # norm/transpose stages, or the in-order PE stream stalls
                # behind the WAR dependency.
                return False
            av_backlog.pop(0)
            av_unit(blk_idx, jt, e, hp, av_t, last)
            return True

        for blk_idx, (hp, ib) in enumerate(BLOCKS):
            i0 = ib * TB
            av_t = ps_av.tile([P, TB], F32, tag="av", name="av_t")
            ps_q = [scores(hp, i0, 0), scores(hp, i0, 1)]
            for jt in range(NT):
                psAB = ps_q.pop(0)
                e = exp_unit(blk_idx, jt, psAB)
                av_backlog.append((blk_idx, jt, e, hp, av_t, jt == NT - 1))
                if jt + 2 < NT:
                    ps_q.append(scores(hp, i0, jt + 2))
                if norm_q:
                    norm_q.pop(0)()
                    if norm_q:
                        norm_q.pop(0)()
                    pop_av()
                else:
                    pop_av()
                    if len(av_backlog) > 10:
                        pop_av()
                    if steps:
                        steps.pop(0)()
            # this block's AVs must all be emitted before its norm stages
            # are queued (cross-engine emission-order discipline).
            while av_popped[blk_idx] < NT:
                if norm_q:
                    norm_q.pop(0)()
                    continue
                if pop_av():
                    continue
                if steps:
                    steps.pop(0)()
                    continue
                raise RuntimeError("scheduling deadlock in block flush")
            norm_q.extend(norm_stages(blk_idx, hp, i0, av_t))

            # schedule out-projection for completed token ranges
            if (hp, ib) == (1, 0):
                steps.extend(outproj_steps(0))
            elif (hp, ib) == (1, 1):
                steps.extend(outproj_steps(1))
            elif (hp, ib) == (1, 2):
                steps.extend(outproj_steps(2))

        # tail: flush norms, then the last token range's out-projection
        while norm_q:
            norm_q.pop(0)()
        while steps:
            steps.pop(0)()
        for step in outproj_steps(3):
            step()

    nc.compile()
    return nc


def _get_nc(mask_any: bool) -> bass.Bass:
    if mask_any not in _CACHE:
        _CACHE[mask_any] = _build(mask_any)
    return _CACHE[mask_any]


def _in_maps(x, mask, Wq, Wkv, Wo, mask_any):
    import ml_dtypes

    bf = ml_dtypes.bfloat16
    maps = []
    # host-side transpose: xt[c, p, t] = x[t, c*128+p]
    xtb = [
        np.ascontiguousarray(
            x[b].T.astype(bf).reshape(ND, P, N)
        )
        for b in range(B)
    ]
    for c in range(8):
        b, r = divmod(c, GROUP)
        m = {
            "xt": xtb[b],
            "wq_s": np.ascontiguousarray(Wq[:, r * IC: (r + 1) * IC].astype(bf)),
            "wk_s": np.ascontiguousarray(Wkv[:, r * IC: (r + 1) * IC].astype(bf)),
            "wv_s": np.ascontiguousarray(
                Wkv[:, INNER + r * IC: INNER + (r + 1) * IC].astype(bf)
            ),
            "wo_s": np.ascontiguousarray(Wo[r * IC: (r + 1) * IC, :].astype(bf)),
        }
        if mask_any:
            mvec = np.where(mask[b], np.float32(NEG), np.float32(0.0)).astype(
                np.float32
            )
            m["mbias"] = np.ascontiguousarray(mvec.reshape(NT, P).T)
        maps.append(m)
    return maps


_RUNNER = {}


def _get_runner(mask_any: bool):
    """Build (once) a cached jax-jitted SPMD executor for the Bass module."""
    if mask_any in _RUNNER:
        return _RUNNER[mask_any]
    import jax
    from jax.sharding import Mesh, PartitionSpec
    from jax.experimental.shard_map import shard_map
    from concourse import bass2jax

    nc = _get_nc(mask_any)
    bass2jax.install_neuronx_cc_hook()

    partition_name = (
        nc.partition_id_tensor.name if nc.partition_id_tensor else None
    )
    in_names, out_names, out_avals = [], [], []
    for alloc in nc.m.functions[0].allocations:
        if not isinstance(alloc, mybir.MemoryLocationSet):
            continue
        name = alloc.memorylocations[0].name
        if alloc.kind == "ExternalInput":
            if name != partition_name:
                in_names.append(name)
        elif alloc.kind == "ExternalOutput":
            shape = tuple(alloc.tensor_shape)
            dtype = mybir.dt.np(alloc.dtype)
            out_names.append(name)
            out_avals.append(jax.core.ShapedArray(shape, dtype))
    n_params = len(in_names)
    n_outs = len(out_avals)
    all_names = list(in_names) + list(out_names)
    if partition_name is not None:
        all_names.append(partition_name)
    donate = tuple(range(n_params, n_params + n_outs))

    def _body(*args):
        operands = list(args)
        if partition_name is not None:
            operands.append(bass2jax.partition_id_tensor())
        outs = bass2jax._bass_exec_p.bind(
            *operands,
            out_avals=tuple(out_avals),
            in_names=tuple(all_names),
            out_names=tuple(out_names),
            lowering_input_output_aliases=(),
            sim_require_finite=True,
            sim_require_nnan=True,
            nc=nc,
        )
        return tuple(outs)

    devices = jax.devices()[:8]
    mesh = Mesh(np.asarray(devices), ("core",))
    in_specs = (PartitionSpec("core"),) * (n_params + n_outs)
    out_specs = (PartitionSpec("core"),) * n_outs
    sharded = jax.jit(
        shard_map(
            _body, mesh=mesh, in_specs=in_specs, out_specs=out_specs,
            check_rep=False,
        ),
        donate_argnums=donate,
        keep_unused=True,
    )
    zero_shapes = [tuple(a.shape) for a in out_avals]
    zero_dtypes = [a.dtype for a in out_avals]

    def call(maps):
        concat_in = [
            np.concatenate([np.asarray(maps[c][nm]) for c in range(8)], axis=0)
            for nm in in_names
        ]
        concat_zeros = [
            np.zeros((8 * s[0], *s[1:]), d)
            for s, d in zip(zero_shapes, zero_dtypes)
        ]
        out_arrs = sharded(*concat_in, *concat_zeros)
        return [
            {
                nm: np.asarray(out_arrs[i]).reshape(8, *zero_shapes[i])[c]
                for i, nm in enumerate(out_names)
            }
            for c in range(8)
        ]

    _RUNNER[mask_any] = call
    return call


def run(x, mask, Wq, Wkv, Wo, bo, trace=False):
    x = np.asarray(x, np.float32)
    mask = np.asarray(mask, bool)
    Wq = np.asarray(Wq, np.float32)
    Wkv = np.asarray(Wkv, np.float32)
    Wo = np.asarray(Wo, np.float32)
    bo = np.asarray(bo, np.float32)
    mask_any = bool(mask.any())
    maps = _in_maps(x, mask, Wq, Wkv, Wo, mask_any)
    results = _get_runner(mask_any)(maps)
    out = np.empty((B, N, DIM), np.float32)
    for b in range(B):
        acc = results[GROUP * b]["y"].copy()
        for r in range(1, GROUP):
            acc += results[GROUP * b + r]["y"]
        out[b] = acc + bo
    return out, results


def kernel(x, mask, Wq, Wkv, Wo, bo):
    out, _ = run(x, mask, Wq, Wkv, Wo, bo, trace=False)
    return out
